# revision 46
# baseline (speedup 1.0000x reference)
"""Trainium2 Bass kernel for nn_BasicBlock_1w1a (binary conv BasicBlock).

Self-contained: takes FULL inputs (batch 64), shards batch across 8 NeuronCores,
runs a single SPMD Bass/Tile kernel with in-kernel AllGathers for the
training-mode BatchNorm batch statistics, gathers the full output.

Per block (twice):
  S      = conv3x3(sign(x), sign(w))        # fp8 DoubleRow matmuls, exact
  gate   = sigmoid(BN_dada(avgpool8(x) @ dw))
  u      = prelu(S * alpha * gate, a)       # gate/alpha folded into BN affine
  out    = BN(u) * g + b + x                # batch stats via AllGather

v6 structure (trace-driven evolution of v3):
  - input DMA split across both HWDGE rings, w1/w2 packed oi-major so the
    first conv group's weight half is one small early DMA; all launches
    up-front (≤7 on the ACT ring so ring credits never head-of-line block
    the first signs)
  - PE warmup dummy-MM chain on a garbage tile right after launch
  - block-1 signs fused per image (both ci planes in one ACT op)
  - dada after conv oi=0 with stats+AllGather-trigger split from the gate
    computation: the AG flies under conv oi=1 (absorbing cross-core launch
    skew) and the gate's sigmoid/affine chain is emitted just before
    main_stats so it never head-of-line blocks conv evictions in the
    ACT/DVE FIFOs
  - pool trees split DVE/GpSimd; GpSimd FIFO kept clear ahead of every
    collective trigger (triggers must fire promptly on all cores or the
    CC phase inherits the skew)
  - mid-block x1 update: fused stt on DVE (ci0) / ACT-scale+GpSimd-add
    (ci1); GpSimd has no pointer-scalar ops
  - rsqrt Newton chains vectorized to [P,2], 2 iterations
  - block-2 u' stored bf16 (bitcast view of ut; no sign consumer, so the
    rounding is safe) — faster 1-input tail scales and bn_stats
  - tail: scale+bias ACT/DVE (fp32 tmp — mixed bf16 adds are slower),
    adds DVE/GpSimd 8/8, per-tile DMA out on both rings
"""
import os
import sys

sys.path.insert(0, "/opt/trn_rl_repo")

import numpy as np
import ml_dtypes

import concourse.bass as bass
import concourse.bacc as bacc
import concourse.tile as tile
import concourse.mybir as mybir
from concourse import bass_utils

P = 128
CI = 2
NIMG = 8
NCORES = 8
H = W = 32
S = H * W
SP = 34 * 34
EPS = 1e-5
MAGIC = 0x5F3759DF
AF = mybir.ActivationFunctionType
ALU = mybir.AluOpType
DT = mybir.dt
X_AXIS = mybir.AxisListType.X

_CACHE = {}


def _build():
    nc = bacc.Bacc("TRN2", target_bir_lowering=False, debug=False,
                   num_devices=NCORES)

    x_in = nc.dram_tensor("x", [NIMG, 256, S], DT.float32, kind="ExternalInput")
    # oi-major so each output-channel half is one contiguous DMA
    w1_in = nc.dram_tensor("w1sb", [2, P, CI, 9, P], DT.float8e4,
                           kind="ExternalInput")
    w2_in = nc.dram_tensor("w2sb", [2, P, CI, 9, P], DT.float8e4,
                           kind="ExternalInput")
    # dada weights split hi/lo bf16: [c_lo, ci, hilo, oi, o_lo]
    dw1_in = nc.dram_tensor("dwt1", [P, CI, 2, 2, P], DT.bfloat16,
                            kind="ExternalInput")
    dw2_in = nc.dram_tensor("dwt2", [P, CI, 2, 2, P], DT.bfloat16,
                            kind="ExternalInput")
    # packed per-channel params: j = 0:alpha 1:a 2:g 3:b 4:dg 5:db -> [P, 6, CI]
    pk1_in = nc.dram_tensor("pk1", [P, 6, CI], DT.float32, kind="ExternalInput")
    pk2_in = nc.dram_tensor("pk2", [P, 6, CI], DT.float32, kind="ExternalInput")
    out_t = nc.dram_tensor("out", [NIMG, 256, S], DT.float32,
                           kind="ExternalOutput")

    with tile.TileContext(nc) as tc:
        with tc.tile_pool(name="big", bufs=1) as big, \
             tc.tile_pool(name="small", bufs=1) as small, \
             tc.tile_pool(name="psum", bufs=8, space="PSUM") as psum_pool, \
             tc.tile_pool(name="tmp", bufs=4) as tmppool, \
             tc.tile_pool(name="poola", bufs=3) as poola_pool, \
             tc.tile_pool(name="dram", bufs=1, space="DRAM") as dram:

            # ---- warmup collective: absorbs ncfw init + SPMD launch skew ----
            wu = small.tile([P, 1], DT.float32, tag="wu")
            nc.gpsimd.memset(wu[:], 1.0)
            wu_i = dram.tile([P, 1], DT.float32, tag="wu_i")
            wu_o = dram.tile([P * NCORES, 1], DT.float32, tag="wu_o")
            nc.sync.dma_start(wu_i[:], wu[:])
            nc.gpsimd.collective_compute(
                "AllGather", ALU.bypass, replica_groups=[list(range(NCORES))],
                ins=[wu_i[:].opt()], outs=[wu_o[:].opt()])

            def ag_start(stat_sb, widx, name):
                """DMA stats to DRAM + AllGather, all on the sync queue.

                The collective trigger is issued from the Sync engine so it
                chains in-order behind the stats DMA (no cross-engine
                semaphore hop) and never sits behind GpSimd pool work."""
                bi = dram.tile([P, 4], DT.float32, tag=f"bi_{name}{widx}")
                bo = dram.tile([P * NCORES, 4], DT.float32,
                               tag=f"bo_{name}{widx}")
                nc.sync.dma_start(bi[:], stat_sb[:])
                nc.gpsimd.collective_compute(
                    "AllGather", ALU.bypass,
                    replica_groups=[list(range(NCORES))],
                    ins=[bi[:].opt()], outs=[bo[:].opt()])
                return bo

            def ag_finish(bo, out_sb, widx, name):
                gath = small.tile([P, NCORES, 4], DT.float32,
                                  tag=f"gth_{name}{widx}")
                nc.sync.dma_start(
                    gath[:], bo[:].rearrange("(r p) c -> p r c", p=P))
                nc.vector.tensor_reduce(out_sb[:],
                                        gath[:].rearrange("p r c -> p c r"),
                                        axis=X_AXIS, op=ALU.add)

            def allreduce_stats(stat_sb, out_sb, widx, name):
                ag_finish(ag_start(stat_sb, widx, name), out_sb, widx, name)

            def u_ag_start(widx, m_s):
                """Per-image u sums/sumsq + dada per-image sums in ONE
                [P,48] AllGather.  No gate dependency, so it triggers
                right at conv end — in parallel with the (possibly
                launch-skew-delayed) y-AllGather instead of after it."""
                upb = small.tile([P, 48], DT.float32, tag=f"upb{widx}")
                bv = bnst[widx][:].rearrange("p c n (g f) -> p c n g f", g=4)
                msv = upb[:, 0:16].rearrange("p (c n) -> p c n", c=2)
                qsv = upb[:, 16:32].rearrange("p (c n) -> p c n", c=2)
                mq = small.tile([P, 2, NIMG, 4], DT.float32, tag=f"uq{widx}")
                m2 = small.tile([P, 2, NIMG], DT.float32, tag=f"u2{widx}")
                # sum(u)/256 per (oi,n); sum(u^2) = 256*sum m^2 + sum M2
                nc.vector.tensor_reduce(msv, bv[:, :, :, :, 1], axis=X_AXIS,
                                        op=ALU.add)
                nc.vector.tensor_mul(mq[:], bv[:, :, :, :, 1],
                                     bv[:, :, :, :, 1])
                nc.vector.tensor_reduce(qsv, mq[:], axis=X_AXIS, op=ALU.add)
                nc.vector.tensor_reduce(m2[:], bv[:, :, :, :, 2], axis=X_AXIS,
                                        op=ALU.add)
                nc.vector.tensor_scalar(qsv, qsv, 256.0, None, ALU.mult)
                nc.vector.tensor_add(qsv, qsv, m2[:])
                nc.vector.tensor_copy(upb[:, 32:48],
                                      m_s[:].rearrange("p c n -> p (c n)"))
                bi = dram.tile([P, 48], DT.float32, tag=f"ubi{widx}")
                bo = dram.tile([P * NCORES, 48], DT.float32,
                               tag=f"ubo{widx}")
                nc.sync.dma_start(bi[:], upb[:])
                nc.gpsimd.collective_compute(
                    "AllGather", ALU.bypass,
                    replica_groups=[list(range(NCORES))],
                    ins=[bi[:].opt()], outs=[bo[:].opt()])
                return bo

            def u_ag_finish(widx, pk, bo, A16, B, ar_u):
                """Gates for all 64 images from the gathered dada sums,
                then the gate-weighted global BN sums."""
                g48 = small.tile([P, NCORES, 48], DT.float32,
                                 tag=f"ug{widx}")
                nc.sync.dma_start(
                    g48[:], bo[:].rearrange("(r p) c -> p r c", p=P))
                sg = small.tile([P, NCORES, NIMG], DT.float32,
                                tag=f"usg{widx}")
                w64 = small.tile([P, NCORES, NIMG], DT.float32,
                                 tag=f"uw{widx}")
                s1 = small.tile([P, 2, 2], DT.float32, tag=f"us1_{widx}")
                for oi in range(2):
                    nc.scalar.activation(
                        sg[:], g48[:, :, 32 + 8 * oi:40 + 8 * oi],
                        AF.Sigmoid, bias=B[:, oi:oi + 1],
                        scale=A16[:, oi:oi + 1])
                    nc.vector.tensor_mul(w64[:], sg[:],
                                         g48[:, :, 8 * oi:8 * oi + 8])
                    nc.vector.tensor_reduce(
                        s1[:, oi, 0:1], w64[:].rearrange("p r n -> p (r n)"),
                        axis=X_AXIS, op=ALU.add)
                    nc.vector.tensor_mul(w64[:], sg[:], sg[:])
                    nc.vector.tensor_mul(
                        w64[:], w64[:], g48[:, :, 16 + 8 * oi:24 + 8 * oi])
                    nc.vector.tensor_reduce(
                        s1[:, oi, 1:2], w64[:].rearrange("p r n -> p (r n)"),
                        axis=X_AXIS, op=ALU.add)
                al2 = small.tile([P, 2], DT.float32, tag=f"ual{widx}")
                nc.vector.tensor_mul(al2[:], pk[:, 0, :], pk[:, 0, :])
                nc.vector.tensor_mul(ar_u[:, 0:2], s1[:, :, 0], pk[:, 0, :])
                nc.vector.tensor_scalar(ar_u[:, 0:2], ar_u[:, 0:2], 256.0,
                                        None, ALU.mult)
                nc.vector.tensor_mul(ar_u[:, 2:4], s1[:, :, 1], al2[:])

            xt = big.tile([P, NIMG, CI, S], DT.float32, tag="xt")
            ut = big.tile([P, 2, NIMG, S], DT.float32, tag="ut")
            # sign pads split per image-pair: keeps the scheduler's fused
            # dependency waits fine-grained
            spads = [big.tile([P, CI, 2, SP], DT.float8e4, tag=f"spad{q}",
                              name=f"spad{q}") for q in range(4)]
            w1sb = big.tile([P, 2, CI, 9, P], DT.float8e4, tag="w1")
            w2sb = big.tile([P, 2, CI, 9, P], DT.float8e4, tag="w2")
            dwt1 = big.tile([P, CI, 2, 2, P], DT.bfloat16, tag="dwt1")
            dwt2 = big.tile([P, CI, 2, 2, P], DT.bfloat16, tag="dwt2")
            pk1 = big.tile([P, 6, CI], DT.float32, tag="pk1")
            pk2 = big.tile([P, 6, CI], DT.float32, tag="pk2")
            # garbage tile for PE warmup matmuls (no aliasing with spads)
            wgarb = big.tile([P, CI, 512], DT.float8e4, tag="wgarb")
            # per-(img,half) BN partials from bn_stats: [oi, n, 4 grp, (c,m,M2)]
            bnst = {
                1: small.tile([P, 2, NIMG, 12], DT.float32, tag="bnst1",
                              name="bnst1"),
                2: small.tile([P, 2, NIMG, 12], DT.float32, tag="bnst2",
                              name="bnst2"),
            }
            # pool sums of u' (block1) for the dada2-pools identity
            pools_u = big.tile([P, 2, NIMG, 16], DT.float32, tag="pu")

            nc.gpsimd.memset(wgarb[:].rearrange("p c s -> p (c s)")
                             .bitcast(DT.int32), 0)
            for q in range(4):
                nc.gpsimd.memset(
                    spads[q][:].rearrange("p c n s -> p (c n s)")
                    .bitcast(DT.int32), 0)

            # ---- PE warmup: ~16 DR matmuls on garbage so HAM hits K=8/8
            # before the first real conv matmul ----
            wps = psum_pool.tile([P, 512], DT.float32, tag="ps", name="wps")
            wg_rhs = wgarb[:].rearrange("p c (r w) -> p c r w", r=16)
            for i in range(16):
                nc.tensor.matmul(wps[:], wgarb[:, :, 0:P], wg_rhs,
                                 start=(i == 0), stop=(i == 15),
                                 perf_mode=mybir.MatmulPerfMode.DoubleRow)
            def xv_of(n):
                return x_in[n].rearrange("(ci p) s -> p ci s", p=P)

            # per-plane DMAs: ci0 planes + the oi=1 weight half on the
            # sync ring; ci1 planes 0-3 + the startup-critical oi=0
            # weight half on the scalar ring (≤5 launches so ring credits
            # never head-of-line block the ACT queue before the signs);
            # ci1 planes 4-7 are launched from inside the sign-group loop
            nc.scalar.dma_start(xt[:, 0, 1, :], xv_of(0)[:, 1, :])
            nc.scalar.dma_start(w1sb[:, 0], w1_in[0])
            nc.sync.dma_start(xt[:, 0, 0, :], xv_of(0)[:, 0, :])
            nc.sync.dma_start(xt[:, 1, 0, :], xv_of(1)[:, 0, :])
            for n in range(1, 4):
                nc.scalar.dma_start(xt[:, n, 1, :], xv_of(n)[:, 1, :])
            for n in range(2, NIMG):
                nc.sync.dma_start(xt[:, n, 0, :], xv_of(n)[:, 0, :])
            nc.sync.dma_start(xt[:, 6, 1, :], xv_of(6)[:, 1, :])
            nc.sync.dma_start(xt[:, 7, 1, :], xv_of(7)[:, 1, :])
            nc.sync.dma_start(w1sb[:, 1], w1_in[1])
            nc.sync.dma_start(pk1[:], pk1_in[:])
            nc.sync.dma_start(dwt1[:], dw1_in[:])
            nc.sync.dma_start(w2sb[:, 0], w2_in[0])
            nc.sync.dma_start(w2sb[:, 1], w2_in[1])
            nc.sync.dma_start(dwt2[:], dw2_in[:])
            nc.sync.dma_start(pk2[:], pk2_in[:])

            def sign_into_spad(n, ci, bias=0.0):
                view = spads[n >> 1][:, ci, n & 1, :].rearrange(
                    "p (r c) -> p r c", r=34)
                nc.scalar.activation(
                    view[:, 1:33, 1:33],
                    xt[:, n, ci, :].rearrange("p (h w) -> p h w", h=H),
                    AF.Sign, bias=bias)

            def sign_img(n):
                """Both ci planes of image n in one ACT op (bias 0 only)."""
                view = spads[n >> 1][:, :, n & 1, :].rearrange(
                    "p ci (r c) -> p ci r c", r=34)
                nc.scalar.activation(
                    view[:, :, 1:33, 1:33],
                    xt[:, n, :, :].rearrange("p ci (h w) -> p ci h w", h=H),
                    AF.Sign)

            def pool_dve(src_ap, dst_16, key):
                """8x8 sum-pool of one [P, 1024] (h,w) plane, DVE 2-stage."""
                pa = poola_pool.tile([P, H * 4], DT.float32, tag="poola",
                                     name=f"pa_{key}")
                nc.vector.tensor_reduce(
                    pa[:],
                    src_ap.rearrange("p (h pw w) -> p h pw w", h=H, pw=4),
                    axis=X_AXIS, op=ALU.add)
                nc.vector.tensor_reduce(
                    dst_16.rearrange("p (ph pw) -> p ph pw", ph=4),
                    pa[:].rearrange("p (ph hh pw) -> p ph pw hh", ph=4, hh=8),
                    axis=X_AXIS, op=ALU.add)

            def pool_tree(src_ap, dst_16, key):
                """Same pool, stage-1 as 3 GpSimd add-tree ops (off-DVE)."""
                t1 = poola_pool.tile([P, 512], DT.float32, tag="poolt1",
                                     name=f"pt1_{key}")
                t2 = poola_pool.tile([P, 256], DT.float32, tag="poolt2",
                                     name=f"pt2_{key}")
                pa = poola_pool.tile([P, H * 4], DT.float32, tag="poola",
                                     name=f"pa_{key}")
                xv = src_ap.rearrange("p (h pw a b) -> p h pw a b", h=H,
                                      pw=4, a=2)
                nc.gpsimd.tensor_add(
                    t1[:].rearrange("p (h pw b) -> p h pw b", h=H, pw=4),
                    xv[:, :, :, 0, :], xv[:, :, :, 1, :])
                t1v = t1[:].rearrange("p (h pw a b) -> p h pw a b", h=H,
                                      pw=4, a=2)
                nc.gpsimd.tensor_add(
                    t2[:].rearrange("p (h pw b) -> p h pw b", h=H, pw=4),
                    t1v[:, :, :, 0, :], t1v[:, :, :, 1, :])
                t2v = t2[:].rearrange("p (h pw a) -> p h pw a", h=H, pw=4)
                nc.gpsimd.tensor_add(
                    pa[:].rearrange("p (h pw) -> p h pw", h=H),
                    t2v[:, :, :, 0], t2v[:, :, :, 1])
                nc.vector.tensor_reduce(
                    dst_16.rearrange("p (ph pw) -> p ph pw", ph=4),
                    pa[:].rearrange("p (ph hh pw) -> p ph pw hh", ph=4, hh=8),
                    axis=X_AXIS, op=ALU.add)

            def rsqrt_inplace(k, t, e1):
                """k = 1/sqrt(t) elementwise, DVE (quake seed + 3 Newton)."""
                ki = k.bitcast(DT.int32)
                nc.vector.tensor_scalar(ki, t.bitcast(DT.int32), 1, None,
                                        ALU.arith_shift_right)
                nc.vector.tensor_scalar(ki, ki, MAGIC, None, ALU.subtract)
                nc.vector.tensor_scalar(ki, ki, -1, None, ALU.mult)
                for _ in range(2):
                    nc.vector.tensor_mul(e1, k, k)
                    nc.vector.tensor_mul(e1, e1, t)
                    nc.vector.tensor_scalar(e1, e1, -0.5, 1.5, ALU.mult,
                                            ALU.add)
                    nc.vector.tensor_mul(k, k, e1)

            p_tiles = {
                1: small.tile([P, CI, NIMG, 16], DT.float32, name="p_t1",
                              tag="p1"),
                2: small.tile([P, CI, NIMG, 16], DT.float32, name="p_t2",
                              tag="p2"),
            }

            # bf16 view of ut for block-2's u' (no sign path downstream, so
            # bf16 rounding is safe; halves the tail SBUF traffic)
            ut_bf = ut[:].bitcast(DT.bfloat16)

            def conv_quad(widx, wsb, pk, oi, imgs, half, do_pool=True):
                """one LDW per kk feeds len(imgs) N=512 DoubleRow matmuls."""
                tl = {n: psum_pool.tile([P, 512], DT.float32, tag="ps",
                                        name=f"ps{widx}_{oi}_{half}_{n}")
                      for n in imgs}
                for kk in range(9):
                    dy, dx = divmod(kk, 3)
                    lhsT = wsb[:, oi, :, kk, :]
                    for j, n in enumerate(imgs):
                        sview = spads[n >> 1][:, :, n & 1, :].rearrange(
                            "p ci (r c) -> p ci r c", r=34)
                        mm = nc.tensor.matmul(
                            tl[n][:], lhsT,
                            sview[:, :, half * 16 + dy:half * 16 + dy + 16,
                                  dx:dx + 32],
                            start=(kk == 0), stop=(kk == 8),
                            perf_mode=mybir.MatmulPerfMode.DoubleRow)
                        if j > 0:
                            # same stationary weights as the j==0 matmul of
                            # this kk — skip the redundant LDWEIGHTS
                            mm.ins.ldweights = False
                for n in imgs:
                    if widx == 1:
                        u_sl = ut[:, oi, n, half * 512:(half + 1) * 512]
                    else:
                        u_sl = ut_bf[:, oi, n, half * 512:(half + 1) * 512]
                    nc.scalar.activation(u_sl, tl[n][:], AF.Prelu,
                                         alpha=pk[:, 1, oi:oi + 1])
                    nc.vector.bn_stats(
                        bnst[widx][:, oi, n, half * 6:(half + 1) * 6], u_sl)
                    if widx == 1 and half == 1 and do_pool:
                        # split u-pools DVE/GpSimd; the y1 trigger precedes
                        # these trees in the GpSimd FIFO (dada1 is emitted
                        # before the oi=1 quads), so it still fires promptly
                        pool_fn = pool_dve if oi == 0 else pool_tree
                        pool_fn(ut[:, oi, n, :], pools_u[:, oi, n, :],
                                f"u_{oi}_{n}")

            def dada_mms(widx, dwt, p_t):
                """hi/lo split + 16 dada matmuls + psum evict -> ysb."""
                ph = small.tile([P, CI, NIMG * 16], DT.bfloat16, tag=f"ph{widx}")
                pl = small.tile([P, CI, NIMG * 16], DT.bfloat16, tag=f"pl{widx}")
                ysb = small.tile([P, 2, NIMG * 16], DT.float32, tag=f"y{widx}")
                p_view = p_t[:].rearrange("p c n s -> p c (n s)")
                if widx == 1:
                    # GpSimd idle-ish during conv1 oi=1
                    nc.gpsimd.tensor_copy(ph[:], p_view)
                    nc.gpsimd.tensor_sub(pl[:], p_view, ph[:])
                else:
                    # mid-phase: GpSimd busy with stt halves; DVE has slack
                    nc.vector.tensor_copy(ph[:], p_view)
                    nc.vector.tensor_sub(pl[:], p_view, ph[:])
                for oi in range(2):
                    psy = psum_pool.tile([P, NIMG * 16], DT.float32,
                                         tag="ps", name=f"psy{widx}_{oi}")
                    terms = [(hl, pp) for hl in range(2) for pp in (ph, pl)]
                    for ci in range(CI):
                        for ti, (hl, pp) in enumerate(terms):
                            nc.tensor.matmul(
                                psy[:], dwt[:, ci, hl, oi, :], pp[:, ci, :],
                                start=(ci == 0 and ti == 0),
                                stop=(ci == CI - 1 and ti == len(terms) - 1))
                    nc.scalar.activation(ysb[:, oi, :], psy[:], AF.Copy)
                return ysb

            def dada_stats_start(widx, ysb, ystat):
                """BN-dada stats from ysb -> AllGather launch."""
                ynst = small.tile([P, 2, 6], DT.float32, tag=f"yn{widx}")
                m_s = small.tile([P, 2, NIMG], DT.float32, tag=f"ms{widx}")
                msq = small.tile([P, 2, 2], DT.float32, tag=f"msq{widx}")
                for oi in range(2):
                    nc.vector.bn_stats(ynst[:, oi, :], ysb[:, oi, :])
                nc.vector.tensor_reduce(
                    m_s[:], ysb[:].rearrange("p c (n q) -> p c n q", n=NIMG),
                    axis=X_AXIS, op=ALU.add)
                yv = ynst[:].rearrange("p c (g f) -> p c g f", g=2)
                # ysum = 64*(m_e + m_o); ysq = M2_e + M2_o + 64*(m_e^2+m_o^2)
                nc.vector.tensor_reduce(ystat[:, 0:2], yv[:, :, :, 1],
                                        axis=X_AXIS, op=ALU.add)
                nc.vector.tensor_scalar(ystat[:, 0:2], ystat[:, 0:2], 64.0,
                                        None, ALU.mult)
                nc.vector.tensor_mul(msq[:], yv[:, :, :, 1], yv[:, :, :, 1])
                nc.vector.tensor_reduce(ystat[:, 2:4], msq[:], axis=X_AXIS,
                                        op=ALU.add)
                nc.vector.tensor_scalar(ystat[:, 2:4], ystat[:, 2:4], 64.0,
                                        None, ALU.mult)
                m2s = small.tile([P, 2], DT.float32, tag=f"m2s{widx}")
                nc.vector.tensor_reduce(m2s[:], yv[:, :, :, 2], axis=X_AXIS,
                                        op=ALU.add)
                nc.vector.tensor_add(ystat[:, 2:4], ystat[:, 2:4], m2s[:])
                return ag_start(ystat, widx, "y"), m_s

            def dada_gate(widx, pk, bo_y, m_s, gate, ar_y):
                """AG result -> gate.  Emitted late (just before it's
                needed) so the affine chain / sigmoid never head-of-line
                block the conv evictions on DVE/ACT."""
                ag_finish(bo_y, ar_y, widx, "y")
                cnt_y = float(NCORES * NIMG * 16)
                t = small.tile([P, 2], DT.float32, tag=f"t{widx}")
                mu = small.tile([P, 2], DT.float32, tag=f"mu{widx}")
                k = small.tile([P, 2], DT.float32, tag=f"k{widx}")
                e1 = small.tile([P, 2], DT.float32, tag=f"e{widx}")
                A16 = small.tile([P, 2], DT.float32, tag=f"A{widx}")
                B = small.tile([P, 2], DT.float32, tag=f"B{widx}")
                nc.vector.tensor_scalar(t[:], ar_y[:, 2:4], 1.0 / cnt_y, EPS,
                                        ALU.mult, ALU.add)
                nc.vector.tensor_scalar(mu[:], ar_y[:, 0:2], 1.0 / cnt_y,
                                        None, ALU.mult)
                nc.vector.tensor_mul(e1[:], mu[:], mu[:])
                nc.vector.tensor_sub(t[:], t[:], e1[:])
                rsqrt_inplace(k[:], t[:], e1[:])
                # A = k*dg; sigmoid(A*(m_s/16) + B): scale = A/16
                nc.vector.tensor_mul(A16[:], k[:], pk[:, 4, :])
                nc.vector.tensor_mul(B[:], mu[:], A16[:])
                nc.vector.tensor_sub(B[:], pk[:, 5, :], B[:])
                nc.vector.tensor_scalar(A16[:], A16[:], 1.0 / 16.0, None,
                                        ALU.mult)
                for oi in range(2):
                    sig = small.tile([P, NIMG], DT.float32,
                                     tag=f"sg{widx}_{oi}")
                    nc.scalar.activation(sig[:], m_s[:, oi, :], AF.Sigmoid,
                                         bias=B[:, oi:oi + 1],
                                         scale=A16[:, oi:oi + 1])
                    nc.vector.tensor_scalar(gate[:, oi, :], sig[:],
                                            pk[:, 0, oi:oi + 1], None, ALU.mult)
                return A16, B

            def main_stats(widx, gate, ustat):
                """usum/usq per image from bn_stats partials, gate-weighted."""
                bv = bnst[widx][:].rearrange("p c n (g f) -> p c n g f", g=4)
                ms = small.tile([P, 2, NIMG], DT.float32, tag=f"us_m{widx}")
                mq = small.tile([P, 2, NIMG, 4], DT.float32, tag=f"us_q{widx}")
                qs = small.tile([P, 2, NIMG], DT.float32, tag=f"us_s{widx}")
                m2 = small.tile([P, 2, NIMG], DT.float32, tag=f"us_2{widx}")
                w8 = small.tile([P, 2, NIMG], DT.float32, tag=f"us_w{widx}")
                g2 = small.tile([P, 2, NIMG], DT.float32, tag=f"us_g{widx}")
                # sum(u) per (oi,n) = 256 * sum of 4 group means
                nc.vector.tensor_reduce(ms[:], bv[:, :, :, :, 1], axis=X_AXIS,
                                        op=ALU.add)
                # sum(u^2) = sum M2 + 256 * sum m^2
                nc.vector.tensor_mul(mq[:], bv[:, :, :, :, 1],
                                     bv[:, :, :, :, 1])
                nc.vector.tensor_reduce(qs[:], mq[:], axis=X_AXIS, op=ALU.add)
                nc.vector.tensor_reduce(m2[:], bv[:, :, :, :, 2], axis=X_AXIS,
                                        op=ALU.add)
                nc.vector.tensor_scalar(qs[:], qs[:], 256.0, None, ALU.mult)
                nc.vector.tensor_add(qs[:], qs[:], m2[:])
                # gate-weighted: sum_n g*usum, sum_n g^2*usq   (256 into scale)
                nc.vector.tensor_mul(w8[:], ms[:], gate[:])
                nc.vector.tensor_reduce(ustat[:, 0:2], w8[:], axis=X_AXIS,
                                        op=ALU.add)
                nc.vector.tensor_scalar(ustat[:, 0:2], ustat[:, 0:2], 256.0,
                                        None, ALU.mult)
                nc.vector.tensor_mul(g2[:], gate[:], gate[:])
                nc.vector.tensor_mul(w8[:], qs[:], g2[:])
                nc.vector.tensor_reduce(ustat[:, 2:4], w8[:], axis=X_AXIS,
                                        op=ALU.add)

            def bn_affine(widx, pk, ar_u, gate, AB, gA):
                """A = k*g, B = b - A*mu, gA[n] = A*gate[n].  [P,2] chain."""
                cnt_u = float(NCORES * NIMG * S)
                t = small.tile([P, 2], DT.float32, tag=f"tu{widx}")
                mu = small.tile([P, 2], DT.float32, tag=f"muu{widx}")
                k = small.tile([P, 2], DT.float32, tag=f"ku{widx}")
                e1 = small.tile([P, 2], DT.float32, tag=f"eu{widx}")
                nc.vector.tensor_scalar(t[:], ar_u[:, 2:4], 1.0 / cnt_u, EPS,
                                        ALU.mult, ALU.add)
                nc.vector.tensor_scalar(mu[:], ar_u[:, 0:2], 1.0 / cnt_u,
                                        None, ALU.mult)
                nc.vector.tensor_mul(e1[:], mu[:], mu[:])
                nc.vector.tensor_sub(t[:], t[:], e1[:])
                rsqrt_inplace(k[:], t[:], e1[:])
                nc.vector.tensor_mul(AB[:, 0, :], k[:], pk[:, 2, :])
                nc.vector.tensor_mul(e1[:], mu[:], AB[:, 0, :])
                nc.vector.tensor_sub(AB[:, 1, :], pk[:, 3, :], e1[:])
                for ci in range(2):
                    nc.vector.tensor_scalar(gA[:, ci, :], gate[:, ci, :],
                                            AB[:, 0, ci:ci + 1], None, ALU.mult)

            gate1 = small.tile([P, 2, NIMG], DT.float32, tag="g1")
            gate2 = small.tile([P, 2, NIMG], DT.float32, tag="g2")
            ystat1 = small.tile([P, 4], DT.float32, tag="ys1")
            ystat2 = small.tile([P, 4], DT.float32, tag="ys2")
            ar_y1 = small.tile([P, 4], DT.float32, tag="ary1")
            ar_y2 = small.tile([P, 4], DT.float32, tag="ary2")
            ustat1 = small.tile([P, 4], DT.float32, tag="us1")
            ustat2 = small.tile([P, 4], DT.float32, tag="us2")
            ar_u1 = small.tile([P, 4], DT.float32, tag="aru1")
            ar_u2 = small.tile([P, 4], DT.float32, tag="aru2")
            AB1 = small.tile([P, 2, 2], DT.float32, tag="ab1")
            AB2 = small.tile([P, 2, 2], DT.float32, tag="ab2")
            gA1 = small.tile([P, 2, NIMG], DT.float32, tag="ga1")
            gA2 = small.tile([P, 2, NIMG], DT.float32, tag="ga2")
            Bp = small.tile([P, 2], DT.float32, tag="bp")

            QUADS = [(0, 1, 2, 3), (4, 5, 6, 7)]

            # ================= block 1 =================
            # first two groups are single-image: start conv as soon as
            # image 0 is signed (input DMA is still streaming), and keep
            # HAM warm straight out of the warmup chain
            for gi, grp in enumerate([(0,), (1,), (2, 3), (4, 5, 6, 7)]):
                for n in grp:
                    sign_img(n)
                    for ci in range(CI):
                        # split pools across DVE / GpSimd-tree
                        pool_fn = pool_dve if ci == 0 else pool_tree
                        pool_fn(xt[:, n, ci, :], p_tiles[1][:, ci, n, :],
                                f"x1_{n}_{ci}")
                if gi == 0:
                    # preload the Sigmoid table while ACT has a hole
                    sgw = small.tile([P, 1], DT.float32, tag="sgw")
                    nc.scalar.activation(sgw[:], wu[:], AF.Sigmoid)
                elif gi == 1:
                    nc.scalar.dma_start(xt[:, 4, 1, :], xv_of(4)[:, 1, :])
                    nc.scalar.dma_start(xt[:, 5, 1, :], xv_of(5)[:, 1, :])
                for half in range(2):
                    conv_quad(1, w1sb, pk1, 0, grp, half)
            # dada right after oi=0: its AllGather flies under oi=1 so
            # gate1 is ready before conv1 ends
            ysb1 = dada_mms(1, dwt1, p_tiles[1])
            bo_y1, m_s1 = dada_stats_start(1, ysb1, ystat1)
            # last groups as image-pairs: the eviction/stats drain ahead
            # of the u-AllGather trigger pipelines ~3us tighter
            for imgs in [QUADS[0], (4, 5), (6, 7)]:
                for half in range(2):
                    conv_quad(1, w1sb, pk1, 1, imgs, half)

            # u-stat AG first (gate-free), then the gate chain: both
            # AllGathers absorb the cross-core skew concurrently
            bo_u1 = u_ag_start(1, m_s1)
            A16_1, B_1 = dada_gate(1, pk1, bo_y1, m_s1, gate1, ar_y1)
            u_ag_finish(1, pk1, bo_u1, A16_1, B_1, ar_u1)
            bn_affine(1, pk1, ar_u1, gate1, AB1, gA1)

            # x1 = gA1[n]*u' + x  (B1 folded into sign bias / final affine)
            # ci0: one fused stt on DVE; ci1: ACT scale + GpSimd add
            def mid_tile(n, ci):
                idx = n * 2 + ci
                if ci == 0:
                    nc.vector.scalar_tensor_tensor(
                        xt[:, n, ci, :], ut[:, ci, n, :],
                        gA1[:, ci, n:n + 1], xt[:, n, ci, :],
                        ALU.mult, ALU.add)
                else:
                    tmp = tmppool.tile([P, S], DT.float32, tag="tmp",
                                       name=f"tmid_{n}_{ci}")
                    nc.scalar.activation(tmp[:], ut[:, ci, n, :],
                                         AF.Identity,
                                         scale=gA1[:, ci, n:n + 1])
                    nc.gpsimd.tensor_add(xt[:, n, ci, :], tmp[:],
                                         xt[:, n, ci, :])
                sign_into_spad(n, ci, bias=AB1[:, 1, ci:ci + 1])

            for n in (0, 1):
                for ci in range(CI):
                    mid_tile(n, ci)

            # dada2 pools via identity: p2 = gA1*pool(u') + pool(x).
            # Emitted after the first two images' mid tiles so conv2 can
            # start; dada2's AllGather then flies under conv2.
            for ci in range(CI):
                for n in range(NIMG):
                    nc.vector.scalar_tensor_tensor(
                        p_tiles[2][:, ci, n, :], pools_u[:, ci, n, :],
                        gA1[:, ci, n:n + 1], p_tiles[1][:, ci, n, :],
                        ALU.mult, ALU.add)
            ysb2 = dada_mms(2, dwt2, p_tiles[2])

            for n in range(2, NIMG):
                for ci in range(CI):
                    mid_tile(n, ci)
            # after the whole mid loop: its DVE/GpSimd ops must not sit
            # behind this chain (or its AllGather trigger) in the FIFOs
            bo_y2, m_s2 = dada_stats_start(2, ysb2, ystat2)

            # ================= block 2 =================
            for grp in [(0, 1), (2, 3), (4, 5, 6, 7)]:
                for half in range(2):
                    conv_quad(2, w2sb, pk2, 0, grp, half)
            for imgs in [QUADS[0], (4, 5), (6, 7)]:
                for half in range(2):
                    conv_quad(2, w2sb, pk2, 1, imgs, half)

            bo_u2 = u_ag_start(2, m_s2)
            A16_2, B_2 = dada_gate(2, pk2, bo_y2, m_s2, gate2, ar_y2)
            u_ag_finish(2, pk2, bo_u2, A16_2, B_2, ar_u2)
            bn_affine(2, pk2, ar_u2, gate2, AB2, gA2)
            # B' = B1 + B2 (skip path carries the un-shifted v = x1 - B1)
            nc.vector.tensor_add(Bp[:], AB1[:, 1, :], AB2[:, 1, :])

            # out = gA2[n]*u' + B' + v ; u' is bf16, tmp bf16 (cuts the
            # 2-input add bandwidth); scales ACT/DVE, adds DVE/GpSimd,
            # out-DMA on two issue queues
            for n in range(NIMG):
                for ci in range(CI):
                    idx = n * 2 + ci
                    ov = out_t[n].rearrange("(ci p) s -> p ci s", p=P)
                    tmp = tmppool.tile([P, S], DT.float32, tag="tmp",
                                       name=f"tout_{n}_{ci}")
                    u2_sl = ut_bf[:, ci, n, 0:S]
                    if idx % 8 < 5:
                        nc.scalar.activation(tmp[:], u2_sl,
                                             AF.Identity,
                                             bias=Bp[:, ci:ci + 1],
                                             scale=gA2[:, ci, n:n + 1])
                    else:
                        nc.vector.tensor_scalar(tmp[:], u2_sl,
                                                gA2[:, ci, n:n + 1],
                                                Bp[:, ci:ci + 1],
                                                ALU.mult, ALU.add)
                    eng = nc.gpsimd if idx % 2 == 0 else nc.vector
                    eng.tensor_add(xt[:, n, ci, :], tmp[:], xt[:, n, ci, :])
                    deng = nc.sync if ci == 0 else nc.scalar
                    deng.dma_start(ov[:, ci, :], xt[:, n, ci, :])

    nc.compile()
    return nc


def _pack_w(w):
    ws = np.sign(w.astype(np.float32))
    t = ws.reshape(2, P, CI, P, 3, 3)           # oi, o_lo, ci, c_lo, dy, dx
    t = t.transpose(0, 3, 2, 4, 5, 1)           # oi, c_lo, ci, dy, dx, o_lo
    return np.ascontiguousarray(t.reshape(2, P, CI, 9, P)).astype(
        ml_dtypes.float8_e4m3)


def _pack_dw(dw):
    d = (dw.astype(np.float32) / 64.0).reshape(2, P, CI, P)  # oi,o_lo,ci,c_lo
    d = d.transpose(3, 2, 0, 1)                               # c_lo,ci,oi,o_lo
    hi = d.astype(ml_dtypes.bfloat16)
    lo = (d - hi.astype(np.float32)).astype(ml_dtypes.bfloat16)
    out = np.empty((P, CI, 2, 2, P), ml_dtypes.bfloat16)
    out[:, :, 0] = hi
    out[:, :, 1] = lo
    return out


def _pack_pk(w, a, g, b, dg, db):
    alpha = np.abs(w.astype(np.float32)).mean(axis=(1, 2, 3))
    fields = [alpha, a, g, b, dg, db]
    pk = np.empty((P, 6, CI), np.float32)
    for j, f in enumerate(fields):
        pk[:, j, :] = np.asarray(f, np.float32).reshape(CI, P).T
    return pk


def kernel(**inputs):
    if "nc" not in _CACHE:
        _CACHE["nc"] = _build()
    nc = _CACHE["nc"]

    x = np.asarray(inputs["x"], np.float32).reshape(64, 256, S)
    feed = {
        "w1sb": _pack_w(np.asarray(inputs["w1"])),
        "w2sb": _pack_w(np.asarray(inputs["w2"])),
        "dwt1": _pack_dw(np.asarray(inputs["dw1"])),
        "dwt2": _pack_dw(np.asarray(inputs["dw2"])),
        "pk1": _pack_pk(np.asarray(inputs["w1"]), inputs["a1"], inputs["g1"],
                        inputs["b1"], inputs["dg1"], inputs["db1"]),
        "pk2": _pack_pk(np.asarray(inputs["w2"]), inputs["a2"], inputs["g2"],
                        inputs["b2"], inputs["dg2"], inputs["db2"]),
    }
    in_maps = []
    for c in range(NCORES):
        m = dict(feed)
        m["x"] = np.ascontiguousarray(x[c * NIMG:(c + 1) * NIMG])
        in_maps.append(m)

    trace = bool(int(os.environ.get("BASS_KERNEL_TRACE", "0")))
    res = bass_utils.run_bass_kernel_spmd(
        nc, in_maps, core_ids=list(range(NCORES)), trace=trace)
    kernel.last_results = res

    out = np.concatenate([res.results[c]["out"] for c in range(NCORES)], axis=0)
    return out.reshape(64, 256, H, W)


# revision 47
# speedup vs baseline: 1.3496x; 1.3496x over previous
"""Trainium2 Bass kernel for nn_BasicBlock_1w1a (binary conv BasicBlock).

Self-contained: takes FULL inputs (batch 64), shards batch across 8 NeuronCores,
runs a single SPMD Bass/Tile kernel with in-kernel AllGathers for the
training-mode BatchNorm batch statistics, gathers the full output.

Per block (twice):
  S      = conv3x3(sign(x), sign(w))        # fp8 DoubleRow matmuls, exact
  gate   = sigmoid(BN_dada(avgpool8(x) @ dw))
  u      = prelu(S * alpha * gate, a)       # gate/alpha folded into BN affine
  out    = BN(u) * g + b + x                # batch stats via AllGather

v6 structure (trace-driven evolution of v3):
  - input DMA split across both HWDGE rings, w1/w2 packed oi-major so the
    first conv group's weight half is one small early DMA; all launches
    up-front (≤7 on the ACT ring so ring credits never head-of-line block
    the first signs)
  - PE warmup dummy-MM chain on a garbage tile right after launch
  - block-1 signs fused per image (both ci planes in one ACT op)
  - dada after conv oi=0 with stats+AllGather-trigger split from the gate
    computation: the AG flies under conv oi=1 (absorbing cross-core launch
    skew) and the gate's sigmoid/affine chain is emitted just before
    main_stats so it never head-of-line blocks conv evictions in the
    ACT/DVE FIFOs
  - pool trees split DVE/GpSimd; GpSimd FIFO kept clear ahead of every
    collective trigger (triggers must fire promptly on all cores or the
    CC phase inherits the skew)
  - mid-block x1 update: fused stt on DVE (ci0) / ACT-scale+GpSimd-add
    (ci1); GpSimd has no pointer-scalar ops
  - rsqrt Newton chains vectorized to [P,2], 2 iterations
  - block-2 u' stored bf16 (bitcast view of ut; no sign consumer, so the
    rounding is safe) — faster 1-input tail scales and bn_stats
  - tail: scale+bias ACT/DVE (fp32 tmp — mixed bf16 adds are slower),
    adds DVE/GpSimd 8/8, per-tile DMA out on both rings
"""
import os
import sys

sys.path.insert(0, "/opt/trn_rl_repo")

import numpy as np
import ml_dtypes

import concourse.bass as bass
import concourse.bacc as bacc
import concourse.tile as tile
import concourse.mybir as mybir
from concourse import bass_utils

P = 128
CI = 2
NIMG = 8
NCORES = 8
H = W = 32
S = H * W
SP = 34 * 34
EPS = 1e-5
MAGIC = 0x5F3759DF
AF = mybir.ActivationFunctionType
ALU = mybir.AluOpType
DT = mybir.dt
X_AXIS = mybir.AxisListType.X

_CACHE = {}


def _build():
    nc = bacc.Bacc("TRN2", target_bir_lowering=False, debug=False,
                   num_devices=NCORES)

    x_in = nc.dram_tensor("x", [NIMG, 256, S], DT.float32, kind="ExternalInput")
    # oi-major so each output-channel half is one contiguous DMA
    w1_in = nc.dram_tensor("w1sb", [2, P, CI, 9, P], DT.float8e4,
                           kind="ExternalInput")
    w2_in = nc.dram_tensor("w2sb", [2, P, CI, 9, P], DT.float8e4,
                           kind="ExternalInput")
    # dada weights split hi/lo bf16: [c_lo, ci, hilo, oi, o_lo]
    dw1_in = nc.dram_tensor("dwt1", [P, CI, 2, 2, P], DT.bfloat16,
                            kind="ExternalInput")
    dw2_in = nc.dram_tensor("dwt2", [P, CI, 2, 2, P], DT.bfloat16,
                            kind="ExternalInput")
    # packed per-channel params: j = 0:alpha 1:a 2:g 3:b 4:dg 5:db -> [P, 6, CI]
    pk1_in = nc.dram_tensor("pk1", [P, 6, CI], DT.float32, kind="ExternalInput")
    pk2_in = nc.dram_tensor("pk2", [P, 6, CI], DT.float32, kind="ExternalInput")
    out_t = nc.dram_tensor("out", [NIMG, 256, S], DT.float32,
                           kind="ExternalOutput")

    with tile.TileContext(nc) as tc:
        with tc.tile_pool(name="big", bufs=1) as big, \
             tc.tile_pool(name="small", bufs=1) as small, \
             tc.tile_pool(name="psum", bufs=8, space="PSUM") as psum_pool, \
             tc.tile_pool(name="tmp", bufs=4) as tmppool, \
             tc.tile_pool(name="poola", bufs=3) as poola_pool, \
             tc.tile_pool(name="dram", bufs=1, space="DRAM") as dram:

            # ---- warmup collective: absorbs ncfw init + SPMD launch skew ----
            wu = small.tile([P, 1], DT.float32, tag="wu")
            nc.gpsimd.memset(wu[:], 1.0)
            wu_i = dram.tile([P, 1], DT.float32, tag="wu_i")
            wu_o = dram.tile([P * NCORES, 1], DT.float32, tag="wu_o")
            nc.sync.dma_start(wu_i[:], wu[:])
            nc.gpsimd.collective_compute(
                "AllGather", ALU.bypass, replica_groups=[list(range(NCORES))],
                ins=[wu_i[:].opt()], outs=[wu_o[:].opt()])

            def ag_start(stat_sb, widx, name):
                """DMA stats to DRAM + AllGather, all on the sync queue.

                The collective trigger is issued from the Sync engine so it
                chains in-order behind the stats DMA (no cross-engine
                semaphore hop) and never sits behind GpSimd pool work."""
                bi = dram.tile([P, 4], DT.float32, tag=f"bi_{name}{widx}")
                bo = dram.tile([P * NCORES, 4], DT.float32,
                               tag=f"bo_{name}{widx}")
                nc.sync.dma_start(bi[:], stat_sb[:])
                nc.gpsimd.collective_compute(
                    "AllGather", ALU.bypass,
                    replica_groups=[list(range(NCORES))],
                    ins=[bi[:].opt()], outs=[bo[:].opt()])
                return bo

            def ag_finish(bo, out_sb, widx, name):
                gath = small.tile([P, NCORES, 4], DT.float32,
                                  tag=f"gth_{name}{widx}")
                nc.sync.dma_start(
                    gath[:], bo[:].rearrange("(r p) c -> p r c", p=P))
                nc.vector.tensor_reduce(out_sb[:],
                                        gath[:].rearrange("p r c -> p c r"),
                                        axis=X_AXIS, op=ALU.add)

            def allreduce_stats(stat_sb, out_sb, widx, name):
                ag_finish(ag_start(stat_sb, widx, name), out_sb, widx, name)

            def u_ag_start(widx, m_s):
                """Per-image u sums/sumsq + dada per-image sums in ONE
                [P,48] AllGather.  No gate dependency, so it triggers
                right at conv end — in parallel with the (possibly
                launch-skew-delayed) y-AllGather instead of after it."""
                upb = small.tile([P, 48], DT.float32, tag=f"upb{widx}")
                bv = bnst[widx][:].rearrange("p c n (g f) -> p c n g f", g=4)
                msv = upb[:, 0:16].rearrange("p (c n) -> p c n", c=2)
                qsv = upb[:, 16:32].rearrange("p (c n) -> p c n", c=2)
                mq = small.tile([P, 2, NIMG, 4], DT.float32, tag=f"uq{widx}")
                m2 = small.tile([P, 2, NIMG], DT.float32, tag=f"u2{widx}")
                # sum(u)/256 per (oi,n); sum(u^2) = 256*sum m^2 + sum M2
                nc.vector.tensor_reduce(msv, bv[:, :, :, :, 1], axis=X_AXIS,
                                        op=ALU.add)
                nc.vector.tensor_mul(mq[:], bv[:, :, :, :, 1],
                                     bv[:, :, :, :, 1])
                nc.vector.tensor_reduce(qsv, mq[:], axis=X_AXIS, op=ALU.add)
                nc.vector.tensor_reduce(m2[:], bv[:, :, :, :, 2], axis=X_AXIS,
                                        op=ALU.add)
                nc.vector.tensor_scalar(qsv, qsv, 256.0, None, ALU.mult)
                nc.vector.tensor_add(qsv, qsv, m2[:])
                nc.vector.tensor_copy(upb[:, 32:48],
                                      m_s[:].rearrange("p c n -> p (c n)"))
                bi = dram.tile([P, 48], DT.float32, tag=f"ubi{widx}")
                bo = dram.tile([P * NCORES, 48], DT.float32,
                               tag=f"ubo{widx}")
                nc.sync.dma_start(bi[:], upb[:])
                nc.gpsimd.collective_compute(
                    "AllGather", ALU.bypass,
                    replica_groups=[list(range(NCORES))],
                    ins=[bi[:].opt()], outs=[bo[:].opt()])
                return bo

            def u_ag_finish(widx, pk, bo, A16, B, ar_u):
                """Gates for all 64 images from the gathered dada sums,
                then the gate-weighted global BN sums."""
                g48 = small.tile([P, NCORES, 48], DT.float32,
                                 tag=f"ug{widx}")
                nc.sync.dma_start(
                    g48[:], bo[:].rearrange("(r p) c -> p r c", p=P))
                sg = small.tile([P, NCORES, NIMG], DT.float32,
                                tag=f"usg{widx}")
                w64 = small.tile([P, NCORES, NIMG], DT.float32,
                                 tag=f"uw{widx}")
                s1 = small.tile([P, 2, 2], DT.float32, tag=f"us1_{widx}")
                for oi in range(2):
                    nc.scalar.activation(
                        sg[:], g48[:, :, 32 + 8 * oi:40 + 8 * oi],
                        AF.Sigmoid, bias=B[:, oi:oi + 1],
                        scale=A16[:, oi:oi + 1])
                    nc.vector.tensor_mul(w64[:], sg[:],
                                         g48[:, :, 8 * oi:8 * oi + 8])
                    nc.vector.tensor_reduce(
                        s1[:, oi, 0:1], w64[:].rearrange("p r n -> p (r n)"),
                        axis=X_AXIS, op=ALU.add)
                    nc.vector.tensor_mul(w64[:], sg[:], sg[:])
                    nc.vector.tensor_mul(
                        w64[:], w64[:], g48[:, :, 16 + 8 * oi:24 + 8 * oi])
                    nc.vector.tensor_reduce(
                        s1[:, oi, 1:2], w64[:].rearrange("p r n -> p (r n)"),
                        axis=X_AXIS, op=ALU.add)
                al2 = small.tile([P, 2], DT.float32, tag=f"ual{widx}")
                nc.vector.tensor_mul(al2[:], pk[:, 0, :], pk[:, 0, :])
                nc.vector.tensor_mul(ar_u[:, 0:2], s1[:, :, 0], pk[:, 0, :])
                nc.vector.tensor_scalar(ar_u[:, 0:2], ar_u[:, 0:2], 256.0,
                                        None, ALU.mult)
                nc.vector.tensor_mul(ar_u[:, 2:4], s1[:, :, 1], al2[:])

            xt = big.tile([P, NIMG, CI, S], DT.float32, tag="xt")
            ut = big.tile([P, 2, NIMG, S], DT.float32, tag="ut")
            # sign pads split per image-pair: keeps the scheduler's fused
            # dependency waits fine-grained
            spads = [big.tile([P, CI, 2, SP], DT.float8e4, tag=f"spad{q}",
                              name=f"spad{q}") for q in range(4)]
            w1sb = big.tile([P, 2, CI, 9, P], DT.float8e4, tag="w1")
            w2sb = big.tile([P, 2, CI, 9, P], DT.float8e4, tag="w2")
            dwt1 = big.tile([P, CI, 2, 2, P], DT.bfloat16, tag="dwt1")
            dwt2 = big.tile([P, CI, 2, 2, P], DT.bfloat16, tag="dwt2")
            pk1 = big.tile([P, 6, CI], DT.float32, tag="pk1")
            pk2 = big.tile([P, 6, CI], DT.float32, tag="pk2")
            # garbage tile for PE warmup matmuls (no aliasing with spads)
            wgarb = big.tile([P, CI, 512], DT.float8e4, tag="wgarb")
            # per-(img,half) BN partials from bn_stats: [oi, n, 4 grp, (c,m,M2)]
            bnst = {
                1: small.tile([P, 2, NIMG, 12], DT.float32, tag="bnst1",
                              name="bnst1"),
                2: small.tile([P, 2, NIMG, 12], DT.float32, tag="bnst2",
                              name="bnst2"),
            }
            # pool sums of u' (block1) for the dada2-pools identity
            pools_u = big.tile([P, 2, NIMG, 16], DT.float32, tag="pu")

            nc.gpsimd.memset(wgarb[:].rearrange("p c s -> p (c s)")
                             .bitcast(DT.int32), 0)
            for q in range(4):
                nc.gpsimd.memset(
                    spads[q][:].rearrange("p c n s -> p (c n s)")
                    .bitcast(DT.int32), 0)

            # ---- PE warmup: ~16 DR matmuls on garbage so HAM hits K=8/8
            # before the first real conv matmul ----
            wps = psum_pool.tile([P, 512], DT.float32, tag="ps", name="wps")
            wg_rhs = wgarb[:].rearrange("p c (r w) -> p c r w", r=16)
            for i in range(16):
                nc.tensor.matmul(wps[:], wgarb[:, :, 0:P], wg_rhs,
                                 start=(i == 0), stop=(i == 15),
                                 perf_mode=mybir.MatmulPerfMode.DoubleRow)
            def xv_of(n):
                return x_in[n].rearrange("(ci p) s -> p ci s", p=P)

            # per-plane DMAs: ci0 planes + the oi=1 weight half on the
            # sync ring; ci1 planes 0-3 + the startup-critical oi=0
            # weight half on the scalar ring (≤5 launches so ring credits
            # never head-of-line block the ACT queue before the signs);
            # ci1 planes 4-7 are launched from inside the sign-group loop
            nc.scalar.dma_start(xt[:, 0, 1, :], xv_of(0)[:, 1, :])
            nc.scalar.dma_start(w1sb[:, 0], w1_in[0])
            nc.sync.dma_start(xt[:, 0, 0, :], xv_of(0)[:, 0, :])
            nc.sync.dma_start(xt[:, 1, 0, :], xv_of(1)[:, 0, :])
            for n in range(1, 4):
                nc.scalar.dma_start(xt[:, n, 1, :], xv_of(n)[:, 1, :])
            for n in range(2, NIMG):
                nc.sync.dma_start(xt[:, n, 0, :], xv_of(n)[:, 0, :])
            nc.sync.dma_start(xt[:, 6, 1, :], xv_of(6)[:, 1, :])
            nc.sync.dma_start(xt[:, 7, 1, :], xv_of(7)[:, 1, :])
            nc.sync.dma_start(w1sb[:, 1], w1_in[1])
            nc.sync.dma_start(pk1[:], pk1_in[:])
            nc.sync.dma_start(dwt1[:], dw1_in[:])
            nc.sync.dma_start(w2sb[:, 0], w2_in[0])
            nc.sync.dma_start(w2sb[:, 1], w2_in[1])
            nc.sync.dma_start(dwt2[:], dw2_in[:])
            nc.sync.dma_start(pk2[:], pk2_in[:])

            def sign_into_spad(n, ci, bias=0.0):
                view = spads[n >> 1][:, ci, n & 1, :].rearrange(
                    "p (r c) -> p r c", r=34)
                nc.scalar.activation(
                    view[:, 1:33, 1:33],
                    xt[:, n, ci, :].rearrange("p (h w) -> p h w", h=H),
                    AF.Sign, bias=bias)

            def sign_img(n):
                """Both ci planes of image n in one ACT op (bias 0 only)."""
                view = spads[n >> 1][:, :, n & 1, :].rearrange(
                    "p ci (r c) -> p ci r c", r=34)
                nc.scalar.activation(
                    view[:, :, 1:33, 1:33],
                    xt[:, n, :, :].rearrange("p ci (h w) -> p ci h w", h=H),
                    AF.Sign)

            def pool_dve(src_ap, dst_16, key):
                """8x8 sum-pool of one [P, 1024] (h,w) plane, DVE 2-stage."""
                pa = poola_pool.tile([P, H * 4], DT.float32, tag="poola",
                                     name=f"pa_{key}")
                nc.vector.tensor_reduce(
                    pa[:],
                    src_ap.rearrange("p (h pw w) -> p h pw w", h=H, pw=4),
                    axis=X_AXIS, op=ALU.add)
                nc.vector.tensor_reduce(
                    dst_16.rearrange("p (ph pw) -> p ph pw", ph=4),
                    pa[:].rearrange("p (ph hh pw) -> p ph pw hh", ph=4, hh=8),
                    axis=X_AXIS, op=ALU.add)

            def pool_tree(src_ap, dst_16, key):
                """Same pool, stage-1 as 3 GpSimd add-tree ops (off-DVE)."""
                t1 = poola_pool.tile([P, 512], DT.float32, tag="poolt1",
                                     name=f"pt1_{key}")
                t2 = poola_pool.tile([P, 256], DT.float32, tag="poolt2",
                                     name=f"pt2_{key}")
                pa = poola_pool.tile([P, H * 4], DT.float32, tag="poola",
                                     name=f"pa_{key}")
                xv = src_ap.rearrange("p (h pw a b) -> p h pw a b", h=H,
                                      pw=4, a=2)
                nc.gpsimd.tensor_add(
                    t1[:].rearrange("p (h pw b) -> p h pw b", h=H, pw=4),
                    xv[:, :, :, 0, :], xv[:, :, :, 1, :])
                t1v = t1[:].rearrange("p (h pw a b) -> p h pw a b", h=H,
                                      pw=4, a=2)
                nc.gpsimd.tensor_add(
                    t2[:].rearrange("p (h pw b) -> p h pw b", h=H, pw=4),
                    t1v[:, :, :, 0, :], t1v[:, :, :, 1, :])
                t2v = t2[:].rearrange("p (h pw a) -> p h pw a", h=H, pw=4)
                nc.gpsimd.tensor_add(
                    pa[:].rearrange("p (h pw) -> p h pw", h=H),
                    t2v[:, :, :, 0], t2v[:, :, :, 1])
                nc.vector.tensor_reduce(
                    dst_16.rearrange("p (ph pw) -> p ph pw", ph=4),
                    pa[:].rearrange("p (ph hh pw) -> p ph pw hh", ph=4, hh=8),
                    axis=X_AXIS, op=ALU.add)

            def rsqrt_inplace(k, t, e1):
                """k = 1/sqrt(t) elementwise, DVE (quake seed + 3 Newton)."""
                ki = k.bitcast(DT.int32)
                nc.vector.tensor_scalar(ki, t.bitcast(DT.int32), 1, None,
                                        ALU.arith_shift_right)
                nc.vector.tensor_scalar(ki, ki, MAGIC, None, ALU.subtract)
                nc.vector.tensor_scalar(ki, ki, -1, None, ALU.mult)
                for _ in range(2):
                    nc.vector.tensor_mul(e1, k, k)
                    nc.vector.tensor_mul(e1, e1, t)
                    nc.vector.tensor_scalar(e1, e1, -0.5, 1.5, ALU.mult,
                                            ALU.add)
                    nc.vector.tensor_mul(k, k, e1)

            p_tiles = {
                1: small.tile([P, CI, NIMG, 16], DT.float32, name="p_t1",
                              tag="p1"),
                2: small.tile([P, CI, NIMG, 16], DT.float32, name="p_t2",
                              tag="p2"),
            }

            # bf16 view of ut for block-2's u' (no sign path downstream, so
            # bf16 rounding is safe; halves the tail SBUF traffic)
            ut_bf = ut[:].bitcast(DT.bfloat16)

            def conv_quad(widx, wsb, pk, oi, imgs, half, do_pool=True):
                """one LDW per kk feeds len(imgs) N=512 DoubleRow matmuls."""
                tl = {n: psum_pool.tile([P, 512], DT.float32, tag="ps",
                                        name=f"ps{widx}_{oi}_{half}_{n}")
                      for n in imgs}
                for kk in range(9):
                    dy, dx = divmod(kk, 3)
                    lhsT = wsb[:, oi, :, kk, :]
                    for j, n in enumerate(imgs):
                        sview = spads[n >> 1][:, :, n & 1, :].rearrange(
                            "p ci (r c) -> p ci r c", r=34)
                        mm = nc.tensor.matmul(
                            tl[n][:], lhsT,
                            sview[:, :, half * 16 + dy:half * 16 + dy + 16,
                                  dx:dx + 32],
                            start=(kk == 0), stop=(kk == 8),
                            perf_mode=mybir.MatmulPerfMode.DoubleRow)
                        if j > 0:
                            # same stationary weights as the j==0 matmul of
                            # this kk — skip the redundant LDWEIGHTS
                            mm.ins.ldweights = False
                for n in imgs:
                    if widx == 1:
                        u_sl = ut[:, oi, n, half * 512:(half + 1) * 512]
                    else:
                        u_sl = ut_bf[:, oi, n, half * 512:(half + 1) * 512]
                    nc.scalar.activation(u_sl, tl[n][:], AF.Prelu,
                                         alpha=pk[:, 1, oi:oi + 1])
                    nc.vector.bn_stats(
                        bnst[widx][:, oi, n, half * 6:(half + 1) * 6], u_sl)
                    if widx == 1 and half == 1 and do_pool:
                        # split u-pools DVE/GpSimd; the y1 trigger precedes
                        # these trees in the GpSimd FIFO (dada1 is emitted
                        # before the oi=1 quads), so it still fires promptly
                        pool_fn = pool_dve if oi == 0 else pool_tree
                        pool_fn(ut[:, oi, n, :], pools_u[:, oi, n, :],
                                f"u_{oi}_{n}")

            def dada_mms(widx, dwt, p_t):
                """hi/lo split + 16 dada matmuls + psum evict -> ysb."""
                ph = small.tile([P, CI, NIMG * 16], DT.bfloat16, tag=f"ph{widx}")
                pl = small.tile([P, CI, NIMG * 16], DT.bfloat16, tag=f"pl{widx}")
                ysb = small.tile([P, 2, NIMG * 16], DT.float32, tag=f"y{widx}")
                p_view = p_t[:].rearrange("p c n s -> p c (n s)")
                if widx == 1:
                    # GpSimd idle-ish during conv1 oi=1
                    nc.gpsimd.tensor_copy(ph[:], p_view)
                    nc.gpsimd.tensor_sub(pl[:], p_view, ph[:])
                else:
                    # mid-phase: GpSimd busy with stt halves; DVE has slack
                    nc.vector.tensor_copy(ph[:], p_view)
                    nc.vector.tensor_sub(pl[:], p_view, ph[:])
                for oi in range(2):
                    psy = psum_pool.tile([P, NIMG * 16], DT.float32,
                                         tag="ps", name=f"psy{widx}_{oi}")
                    terms = [(hl, pp) for hl in range(2) for pp in (ph, pl)]
                    for ci in range(CI):
                        for ti, (hl, pp) in enumerate(terms):
                            nc.tensor.matmul(
                                psy[:], dwt[:, ci, hl, oi, :], pp[:, ci, :],
                                start=(ci == 0 and ti == 0),
                                stop=(ci == CI - 1 and ti == len(terms) - 1))
                    nc.scalar.activation(ysb[:, oi, :], psy[:], AF.Copy)
                return ysb

            def dada_stats_start(widx, ysb, ystat):
                """BN-dada stats from ysb -> AllGather launch."""
                ynst = small.tile([P, 2, 6], DT.float32, tag=f"yn{widx}")
                m_s = small.tile([P, 2, NIMG], DT.float32, tag=f"ms{widx}")
                msq = small.tile([P, 2, 2], DT.float32, tag=f"msq{widx}")
                for oi in range(2):
                    nc.vector.bn_stats(ynst[:, oi, :], ysb[:, oi, :])
                nc.vector.tensor_reduce(
                    m_s[:], ysb[:].rearrange("p c (n q) -> p c n q", n=NIMG),
                    axis=X_AXIS, op=ALU.add)
                yv = ynst[:].rearrange("p c (g f) -> p c g f", g=2)
                # ysum = 64*(m_e + m_o); ysq = M2_e + M2_o + 64*(m_e^2+m_o^2)
                nc.vector.tensor_reduce(ystat[:, 0:2], yv[:, :, :, 1],
                                        axis=X_AXIS, op=ALU.add)
                nc.vector.tensor_scalar(ystat[:, 0:2], ystat[:, 0:2], 64.0,
                                        None, ALU.mult)
                nc.vector.tensor_mul(msq[:], yv[:, :, :, 1], yv[:, :, :, 1])
                nc.vector.tensor_reduce(ystat[:, 2:4], msq[:], axis=X_AXIS,
                                        op=ALU.add)
                nc.vector.tensor_scalar(ystat[:, 2:4], ystat[:, 2:4], 64.0,
                                        None, ALU.mult)
                m2s = small.tile([P, 2], DT.float32, tag=f"m2s{widx}")
                nc.vector.tensor_reduce(m2s[:], yv[:, :, :, 2], axis=X_AXIS,
                                        op=ALU.add)
                nc.vector.tensor_add(ystat[:, 2:4], ystat[:, 2:4], m2s[:])
                return ag_start(ystat, widx, "y"), m_s

            def dada_gate(widx, pk, bo_y, m_s, gate, ar_y):
                """AG result -> gate.  Emitted late (just before it's
                needed) so the affine chain / sigmoid never head-of-line
                block the conv evictions on DVE/ACT."""
                ag_finish(bo_y, ar_y, widx, "y")
                cnt_y = float(NCORES * NIMG * 16)
                t = small.tile([P, 2], DT.float32, tag=f"t{widx}")
                mu = small.tile([P, 2], DT.float32, tag=f"mu{widx}")
                k = small.tile([P, 2], DT.float32, tag=f"k{widx}")
                e1 = small.tile([P, 2], DT.float32, tag=f"e{widx}")
                A16 = small.tile([P, 2], DT.float32, tag=f"A{widx}")
                B = small.tile([P, 2], DT.float32, tag=f"B{widx}")
                nc.vector.tensor_scalar(t[:], ar_y[:, 2:4], 1.0 / cnt_y, EPS,
                                        ALU.mult, ALU.add)
                nc.vector.tensor_scalar(mu[:], ar_y[:, 0:2], 1.0 / cnt_y,
                                        None, ALU.mult)
                nc.vector.tensor_mul(e1[:], mu[:], mu[:])
                nc.vector.tensor_sub(t[:], t[:], e1[:])
                rsqrt_inplace(k[:], t[:], e1[:])
                # A = k*dg; sigmoid(A*(m_s/16) + B): scale = A/16
                nc.vector.tensor_mul(A16[:], k[:], pk[:, 4, :])
                nc.vector.tensor_mul(B[:], mu[:], A16[:])
                nc.vector.tensor_sub(B[:], pk[:, 5, :], B[:])
                nc.vector.tensor_scalar(A16[:], A16[:], 1.0 / 16.0, None,
                                        ALU.mult)
                for oi in range(2):
                    sig = small.tile([P, NIMG], DT.float32,
                                     tag=f"sg{widx}_{oi}")
                    nc.scalar.activation(sig[:], m_s[:, oi, :], AF.Sigmoid,
                                         bias=B[:, oi:oi + 1],
                                         scale=A16[:, oi:oi + 1])
                    nc.vector.tensor_scalar(gate[:, oi, :], sig[:],
                                            pk[:, 0, oi:oi + 1], None, ALU.mult)
                return A16, B

            def main_stats(widx, gate, ustat):
                """usum/usq per image from bn_stats partials, gate-weighted."""
                bv = bnst[widx][:].rearrange("p c n (g f) -> p c n g f", g=4)
                ms = small.tile([P, 2, NIMG], DT.float32, tag=f"us_m{widx}")
                mq = small.tile([P, 2, NIMG, 4], DT.float32, tag=f"us_q{widx}")
                qs = small.tile([P, 2, NIMG], DT.float32, tag=f"us_s{widx}")
                m2 = small.tile([P, 2, NIMG], DT.float32, tag=f"us_2{widx}")
                w8 = small.tile([P, 2, NIMG], DT.float32, tag=f"us_w{widx}")
                g2 = small.tile([P, 2, NIMG], DT.float32, tag=f"us_g{widx}")
                # sum(u) per (oi,n) = 256 * sum of 4 group means
                nc.vector.tensor_reduce(ms[:], bv[:, :, :, :, 1], axis=X_AXIS,
                                        op=ALU.add)
                # sum(u^2) = sum M2 + 256 * sum m^2
                nc.vector.tensor_mul(mq[:], bv[:, :, :, :, 1],
                                     bv[:, :, :, :, 1])
                nc.vector.tensor_reduce(qs[:], mq[:], axis=X_AXIS, op=ALU.add)
                nc.vector.tensor_reduce(m2[:], bv[:, :, :, :, 2], axis=X_AXIS,
                                        op=ALU.add)
                nc.vector.tensor_scalar(qs[:], qs[:], 256.0, None, ALU.mult)
                nc.vector.tensor_add(qs[:], qs[:], m2[:])
                # gate-weighted: sum_n g*usum, sum_n g^2*usq   (256 into scale)
                nc.vector.tensor_mul(w8[:], ms[:], gate[:])
                nc.vector.tensor_reduce(ustat[:, 0:2], w8[:], axis=X_AXIS,
                                        op=ALU.add)
                nc.vector.tensor_scalar(ustat[:, 0:2], ustat[:, 0:2], 256.0,
                                        None, ALU.mult)
                nc.vector.tensor_mul(g2[:], gate[:], gate[:])
                nc.vector.tensor_mul(w8[:], qs[:], g2[:])
                nc.vector.tensor_reduce(ustat[:, 2:4], w8[:], axis=X_AXIS,
                                        op=ALU.add)

            def bn_affine(widx, pk, ar_u, gate, AB, gA):
                """A = k*g, B = b - A*mu, gA[n] = A*gate[n].  [P,2] chain."""
                cnt_u = float(NCORES * NIMG * S)
                t = small.tile([P, 2], DT.float32, tag=f"tu{widx}")
                mu = small.tile([P, 2], DT.float32, tag=f"muu{widx}")
                k = small.tile([P, 2], DT.float32, tag=f"ku{widx}")
                e1 = small.tile([P, 2], DT.float32, tag=f"eu{widx}")
                nc.vector.tensor_scalar(t[:], ar_u[:, 2:4], 1.0 / cnt_u, EPS,
                                        ALU.mult, ALU.add)
                nc.vector.tensor_scalar(mu[:], ar_u[:, 0:2], 1.0 / cnt_u,
                                        None, ALU.mult)
                nc.vector.tensor_mul(e1[:], mu[:], mu[:])
                nc.vector.tensor_sub(t[:], t[:], e1[:])
                rsqrt_inplace(k[:], t[:], e1[:])
                nc.vector.tensor_mul(AB[:, 0, :], k[:], pk[:, 2, :])
                nc.vector.tensor_mul(e1[:], mu[:], AB[:, 0, :])
                nc.vector.tensor_sub(AB[:, 1, :], pk[:, 3, :], e1[:])
                for ci in range(2):
                    nc.vector.tensor_scalar(gA[:, ci, :], gate[:, ci, :],
                                            AB[:, 0, ci:ci + 1], None, ALU.mult)

            gate1 = small.tile([P, 2, NIMG], DT.float32, tag="g1")
            gate2 = small.tile([P, 2, NIMG], DT.float32, tag="g2")
            ystat1 = small.tile([P, 4], DT.float32, tag="ys1")
            ystat2 = small.tile([P, 4], DT.float32, tag="ys2")
            ar_y1 = small.tile([P, 4], DT.float32, tag="ary1")
            ar_y2 = small.tile([P, 4], DT.float32, tag="ary2")
            ustat1 = small.tile([P, 4], DT.float32, tag="us1")
            ustat2 = small.tile([P, 4], DT.float32, tag="us2")
            ar_u1 = small.tile([P, 4], DT.float32, tag="aru1")
            ar_u2 = small.tile([P, 4], DT.float32, tag="aru2")
            AB1 = small.tile([P, 2, 2], DT.float32, tag="ab1")
            AB2 = small.tile([P, 2, 2], DT.float32, tag="ab2")
            gA1 = small.tile([P, 2, NIMG], DT.float32, tag="ga1")
            gA2 = small.tile([P, 2, NIMG], DT.float32, tag="ga2")
            Bp = small.tile([P, 2], DT.float32, tag="bp")

            QUADS = [(0, 1, 2, 3), (4, 5, 6, 7)]

            # ================= block 1 =================
            # first two groups are single-image: start conv as soon as
            # image 0 is signed (input DMA is still streaming), and keep
            # HAM warm straight out of the warmup chain
            for gi, grp in enumerate([(0,), (1,), (2, 3), (4, 5, 6, 7)]):
                for n in grp:
                    sign_img(n)
                    for ci in range(CI):
                        # split pools across DVE / GpSimd-tree
                        pool_fn = pool_dve if ci == 0 else pool_tree
                        pool_fn(xt[:, n, ci, :], p_tiles[1][:, ci, n, :],
                                f"x1_{n}_{ci}")
                if gi == 0:
                    # preload the Sigmoid table while ACT has a hole
                    sgw = small.tile([P, 1], DT.float32, tag="sgw")
                    nc.scalar.activation(sgw[:], wu[:], AF.Sigmoid)
                elif gi == 1:
                    nc.scalar.dma_start(xt[:, 4, 1, :], xv_of(4)[:, 1, :])
                    nc.scalar.dma_start(xt[:, 5, 1, :], xv_of(5)[:, 1, :])
                for half in range(2):
                    conv_quad(1, w1sb, pk1, 0, grp, half)
            # dada right after oi=0: its AllGather flies under oi=1 so
            # gate1 is ready before conv1 ends
            ysb1 = dada_mms(1, dwt1, p_tiles[1])
            bo_y1, m_s1 = dada_stats_start(1, ysb1, ystat1)
            for imgs in QUADS:
                for half in range(2):
                    conv_quad(1, w1sb, pk1, 1, imgs, half)

            # u-stat AG first (gate-free), then the gate chain: both
            # AllGathers absorb the cross-core skew concurrently
            bo_u1 = u_ag_start(1, m_s1)
            A16_1, B_1 = dada_gate(1, pk1, bo_y1, m_s1, gate1, ar_y1)
            u_ag_finish(1, pk1, bo_u1, A16_1, B_1, ar_u1)
            bn_affine(1, pk1, ar_u1, gate1, AB1, gA1)

            # x1 = gA1[n]*u' + x  (B1 folded into sign bias / final affine)
            # ci0: one fused stt on DVE; ci1: ACT scale + GpSimd add
            def mid_tile(n, ci):
                idx = n * 2 + ci
                if ci == 0:
                    nc.vector.scalar_tensor_tensor(
                        xt[:, n, ci, :], ut[:, ci, n, :],
                        gA1[:, ci, n:n + 1], xt[:, n, ci, :],
                        ALU.mult, ALU.add)
                else:
                    tmp = tmppool.tile([P, S], DT.float32, tag="tmp",
                                       name=f"tmid_{n}_{ci}")
                    nc.scalar.activation(tmp[:], ut[:, ci, n, :],
                                         AF.Identity,
                                         scale=gA1[:, ci, n:n + 1])
                    nc.gpsimd.tensor_add(xt[:, n, ci, :], tmp[:],
                                         xt[:, n, ci, :])
                sign_into_spad(n, ci, bias=AB1[:, 1, ci:ci + 1])

            for n in (0, 1):
                for ci in range(CI):
                    mid_tile(n, ci)

            # dada2 pools via identity: p2 = gA1*pool(u') + pool(x).
            # Emitted after the first two images' mid tiles so conv2 can
            # start; dada2's AllGather then flies under conv2.
            for ci in range(CI):
                for n in range(NIMG):
                    nc.vector.scalar_tensor_tensor(
                        p_tiles[2][:, ci, n, :], pools_u[:, ci, n, :],
                        gA1[:, ci, n:n + 1], p_tiles[1][:, ci, n, :],
                        ALU.mult, ALU.add)
            ysb2 = dada_mms(2, dwt2, p_tiles[2])

            for n in range(2, NIMG):
                for ci in range(CI):
                    mid_tile(n, ci)
            # after the whole mid loop: its DVE/GpSimd ops must not sit
            # behind this chain (or its AllGather trigger) in the FIFOs
            bo_y2, m_s2 = dada_stats_start(2, ysb2, ystat2)

            # ================= block 2 =================
            for grp in [(0, 1), (2, 3), (4, 5, 6, 7)]:
                for half in range(2):
                    conv_quad(2, w2sb, pk2, 0, grp, half)
            for imgs in QUADS:
                for half in range(2):
                    conv_quad(2, w2sb, pk2, 1, imgs, half)

            bo_u2 = u_ag_start(2, m_s2)
            A16_2, B_2 = dada_gate(2, pk2, bo_y2, m_s2, gate2, ar_y2)
            u_ag_finish(2, pk2, bo_u2, A16_2, B_2, ar_u2)
            bn_affine(2, pk2, ar_u2, gate2, AB2, gA2)
            # B' = B1 + B2 (skip path carries the un-shifted v = x1 - B1)
            nc.vector.tensor_add(Bp[:], AB1[:, 1, :], AB2[:, 1, :])

            # out = gA2[n]*u' + B' + v ; u' is bf16, tmp bf16 (cuts the
            # 2-input add bandwidth); scales ACT/DVE, adds DVE/GpSimd,
            # out-DMA on two issue queues
            for n in range(NIMG):
                for ci in range(CI):
                    idx = n * 2 + ci
                    ov = out_t[n].rearrange("(ci p) s -> p ci s", p=P)
                    tmp = tmppool.tile([P, S], DT.float32, tag="tmp",
                                       name=f"tout_{n}_{ci}")
                    u2_sl = ut_bf[:, ci, n, 0:S]
                    if idx % 8 < 5:
                        nc.scalar.activation(tmp[:], u2_sl,
                                             AF.Identity,
                                             bias=Bp[:, ci:ci + 1],
                                             scale=gA2[:, ci, n:n + 1])
                    else:
                        nc.vector.tensor_scalar(tmp[:], u2_sl,
                                                gA2[:, ci, n:n + 1],
                                                Bp[:, ci:ci + 1],
                                                ALU.mult, ALU.add)
                    eng = nc.gpsimd if idx % 2 == 0 else nc.vector
                    eng.tensor_add(xt[:, n, ci, :], tmp[:], xt[:, n, ci, :])
                    deng = nc.sync if ci == 0 else nc.scalar
                    deng.dma_start(ov[:, ci, :], xt[:, n, ci, :])

    nc.compile()
    return nc


def _pack_w(w):
    ws = np.sign(w.astype(np.float32))
    t = ws.reshape(2, P, CI, P, 3, 3)           # oi, o_lo, ci, c_lo, dy, dx
    t = t.transpose(0, 3, 2, 4, 5, 1)           # oi, c_lo, ci, dy, dx, o_lo
    return np.ascontiguousarray(t.reshape(2, P, CI, 9, P)).astype(
        ml_dtypes.float8_e4m3)


def _pack_dw(dw):
    d = (dw.astype(np.float32) / 64.0).reshape(2, P, CI, P)  # oi,o_lo,ci,c_lo
    d = d.transpose(3, 2, 0, 1)                               # c_lo,ci,oi,o_lo
    hi = d.astype(ml_dtypes.bfloat16)
    lo = (d - hi.astype(np.float32)).astype(ml_dtypes.bfloat16)
    out = np.empty((P, CI, 2, 2, P), ml_dtypes.bfloat16)
    out[:, :, 0] = hi
    out[:, :, 1] = lo
    return out


def _pack_pk(w, a, g, b, dg, db):
    alpha = np.abs(w.astype(np.float32)).mean(axis=(1, 2, 3))
    fields = [alpha, a, g, b, dg, db]
    pk = np.empty((P, 6, CI), np.float32)
    for j, f in enumerate(fields):
        pk[:, j, :] = np.asarray(f, np.float32).reshape(CI, P).T
    return pk


def kernel(**inputs):
    if "nc" not in _CACHE:
        _CACHE["nc"] = _build()
    nc = _CACHE["nc"]

    x = np.asarray(inputs["x"], np.float32).reshape(64, 256, S)
    feed = {
        "w1sb": _pack_w(np.asarray(inputs["w1"])),
        "w2sb": _pack_w(np.asarray(inputs["w2"])),
        "dwt1": _pack_dw(np.asarray(inputs["dw1"])),
        "dwt2": _pack_dw(np.asarray(inputs["dw2"])),
        "pk1": _pack_pk(np.asarray(inputs["w1"]), inputs["a1"], inputs["g1"],
                        inputs["b1"], inputs["dg1"], inputs["db1"]),
        "pk2": _pack_pk(np.asarray(inputs["w2"]), inputs["a2"], inputs["g2"],
                        inputs["b2"], inputs["dg2"], inputs["db2"]),
    }
    in_maps = []
    for c in range(NCORES):
        m = dict(feed)
        m["x"] = np.ascontiguousarray(x[c * NIMG:(c + 1) * NIMG])
        in_maps.append(m)

    trace = bool(int(os.environ.get("BASS_KERNEL_TRACE", "0")))
    res = bass_utils.run_bass_kernel_spmd(
        nc, in_maps, core_ids=list(range(NCORES)), trace=trace)
    kernel.last_results = res

    out = np.concatenate([res.results[c]["out"] for c in range(NCORES)], axis=0)
    return out.reshape(64, 256, H, W)
